# revision 33
# baseline (speedup 1.0000x reference)
"""Block-sparse attention (block-local) Bass kernel for 8 Trainium2 NeuronCores.

Problem: x[4, 4096, 1024] -> 4 linear projections (Q/K/V/O) + block-local
attention (block size 128, 16 heads, d_k 64), all f32.

Sharding: pure data parallel over tokens. Attention is block-local with
block size 128, so the flattened token axis [16384] splits across 8 cores
into 2048-token shards (16 blocks each) with zero cross-core communication.

Per-core kernel layout (v2 — PE-minimal):
 - x host-transposed/packed as xt [128, 8, 2048] so activations live in SBUF
   with d_model on partitions; weights host-packed [128, 8, 1024] so every
   weight DMA is contiguous 2KB-per-partition lines.
 - Q^T/K^T produced in [d_model, token] layout (what scores matmuls need)
   and drained on the Scalar engine (activation Identity, per-partition bias,
   fused 1/sqrt(dk) scale on Q). V in natural [token, d_model] layout,
   drained on DVE with broadcast bias add.
 - Per 128-token block: scores -> exp (Scalar) -> row-sum/recip/normalize
   (DVE, along free dim) -> A^T via XBAR DMA transpose (no PE, no PSUM) ->
   A@V -> PSUM->SBUF head repack (Scalar) -> Wo projection. Output bias bo
   is added during the DVE drain of the Wo PSUM (no ones-matmul).
 - Software pipeline: scores/softmax for block B+1 are emitted one block
   early, and Wo(B-1) runs between scores(B+1) and A@V(B), so the
   exp->normalize->transpose chain always has a full block of PE work
   (~5us) to hide under.
 - PSUM budget: 2 banks projections/Wo + 4 banks scores + 2 banks A@V = 8.
"""
import sys

if '/opt/trn_rl_repo' not in sys.path:
    sys.path.insert(0, '/opt/trn_rl_repo')

import numpy as np

import concourse.bass as bass
import concourse.mybir as mybir
import concourse.tile as tile
from concourse.vector_clock import ScopedClock
from concourse.bass_utils import run_bass_kernel_spmd

F32 = mybir.dt.float32
F16 = mybir.dt.float16  # attention-path dtype (fp16: same PE rate, more mantissa)
F8 = mybir.dt.float8e4

import os
# Q/K projections in fp8-e4m3 DoubleRow (2 k-rows/partition, ~1.4x matmul
# rate). End-to-end rel err ~1.7e-2 (HW-measured) vs the 2e-2 gate: the
# softmax damps score quantization, and V/O stay fp16 so value-path
# precision is preserved.
QK_FP8 = os.environ.get('QK_FP8', '1') == '1'

D = 1024          # d_model
NH = 16           # heads
DK = 64           # head dim
BS = 128          # attention block size
N_CORES = 8
TOK = 2048        # tokens per core
ST = 512          # supertile tokens
NST = TOK // ST   # supertiles per core
NBLK = TOK // BS  # 16 attention blocks per core
SCALE = 1.0 / 8.0  # 1/sqrt(DK)

_MAX_DRAIN_WAITS = 1


class _SplitDrainTileContext(tile.TileContext):
    """The walrus in this container rejects >1 sync-wait on a NO_STRUCT
    instruction; Tile's exit drain waits on the whole global clock. Spread
    the waits across a chain of drains."""

    def _drain_and_barrier(self, tick_clock, wait_clock):
        nc = self.nc
        probe = nc.sync.drain()
        wait_clock.add_sem_waits(probe.ins, ScopedClock({None: tick_clock.global_clock}))
        si = probe.ins.sync_info
        waits = list(si.on_wait) if (si and si.on_wait) else []
        if len(waits) > _MAX_DRAIN_WAITS:
            probe.ins.sync_info = mybir.SyncInfo(
                on_wait=waits[:_MAX_DRAIN_WAITS],
                on_update=list(si.on_update) if si.on_update else [],
            )
            for i in range(_MAX_DRAIN_WAITS, len(waits), _MAX_DRAIN_WAITS):
                d = nc.sync.drain()
                d.ins.sync_info = mybir.SyncInfo(
                    on_wait=waits[i:i + _MAX_DRAIN_WAITS], on_update=[]
                )
        nc.all_engine_barrier()
        assert self.sems is not None
        popped = nc._tile_sem_poison_stack.pop()
        assert popped is self._sem_poison
        nc.clear_and_free_semaphores(list(self.sems.allocated().values()))
        nc.all_engine_barrier()


def _split_excess_waits(nc, limit=1):
    """The nix walrus rejects instructions carrying more than `limit` sync
    waits. Hoist excess waits onto EventSemaphore instructions inserted just
    before, on the same (in-order) engine — semantics preserved."""
    n_split = 0
    for f in nc.m.functions:
        for bb in f.blocks:
            new = []
            changed = False
            for inst in bb.instructions:
                si = inst.sync_info
                waits = list(si.on_wait) if (si and si.on_wait) else []
                if len(waits) > limit:
                    excess = waits[:-limit]
                    for i in range(0, len(excess), limit):
                        ev = mybir.InstEventSemaphore(
                            name=f'I-splitw-{nc.next_id()}')
                        ev.engine = inst.engine
                        ev.sync_info = mybir.SyncInfo(
                            on_wait=excess[i:i + limit], on_update=[])
                        new.append(ev)
                        n_split += 1
                    inst.sync_info = mybir.SyncInfo(
                        on_wait=waits[-limit:],
                        on_update=list(si.on_update) if si.on_update else [])
                    changed = True
                new.append(inst)
            if changed:
                bb.instructions = new
    return n_split


def build_bass(split_waits=True):
    nc = bass.Bass('TRN2', target_bir_lowering=False, num_devices=N_CORES)

    xt_d = nc.dram_tensor('xt', [128, 8, TOK], F16, kind='ExternalInput')
    if QK_FP8:
        wq_d = nc.dram_tensor('wq', [128, 4, 2, D], F8, kind='ExternalInput')
        wk_d = nc.dram_tensor('wk', [128, 4, 2, D], F8, kind='ExternalInput')
        x8_d = nc.dram_tensor('x8', [128, 4, 2, TOK], F8, kind='ExternalInput')
    else:
        wq_d = nc.dram_tensor('wq', [128, 8, D], F16, kind='ExternalInput')
        wk_d = nc.dram_tensor('wk', [128, 8, D], F16, kind='ExternalInput')
        x8_d = None
    wv_d = nc.dram_tensor('wv', [128, 8, D], F16, kind='ExternalInput')
    wo_d = nc.dram_tensor('wo', [128, 8, D], F16, kind='ExternalInput')
    warm_d = nc.dram_tensor('warm', [128, 512], F16, kind='ExternalInput')
    bq_d = nc.dram_tensor('bq', [128, 8], F32, kind='ExternalInput')  # pre-scaled
    bk_d = nc.dram_tensor('bk', [128, 8], F32, kind='ExternalInput')
    bv_d = nc.dram_tensor('bv', [1, D], F32, kind='ExternalInput')
    bo_d = nc.dram_tensor('bo', [1, D], F32, kind='ExternalInput')
    out_d = nc.dram_tensor('out', [TOK, D], F32, kind='ExternalOutput')

    with _SplitDrainTileContext(nc) as tc:
        _build_body(nc, tc, xt_d, wq_d, wk_d, wv_d, wo_d,
                    bq_d, bk_d, bv_d, bo_d, out_d, x8_d, warm_d)
    if split_waits:
        # CoreSim chokes on the inserted EventSemaphores; only split for HW.
        _split_excess_waits(nc, limit=1)
    return nc


def _build_body(nc, tc, xt_d, wq_d, wk_d, wv_d, wo_d, bq_d, bk_d, bv_d, bo_d,
                out_d, x8_d=None, warm_d=None):
    from contextlib import ExitStack
    with ExitStack() as ctx:
        _build_pools_and_body(nc, tc, ctx, xt_d, wq_d, wk_d, wv_d, wo_d,
                              bq_d, bk_d, bv_d, bo_d, out_d, x8_d, warm_d)


def _build_pools_and_body(nc, tc, ctx, xt_d, wq_d, wk_d, wv_d, wo_d,
                          bq_d, bk_d, bv_d, bo_d, out_d, x8_d=None, warm_d=None):
    AF = mybir.ActivationFunctionType
    OP = mybir.AluOpType
    AX = mybir.AxisListType

    wpool = ctx.enter_context(tc.tile_pool(name='w', bufs=1))
    cpool = ctx.enter_context(tc.tile_pool(name='c', bufs=1))
    xpool = ctx.enter_context(tc.tile_pool(name='x', bufs=1))
    qkv = ctx.enter_context(tc.tile_pool(name='qkv', bufs=2))
    epool = ctx.enter_context(tc.tile_pool(name='e', bufs=3))
    atpool = ctx.enter_context(tc.tile_pool(name='at', bufs=3))
    otpool = ctx.enter_context(tc.tile_pool(name='ot', bufs=2))
    opool = ctx.enter_context(tc.tile_pool(name='o', bufs=2))

    pp = ctx.enter_context(tc.tile_pool(name='pp', bufs=2, space='PSUM'))
    psc = ctx.enter_context(tc.tile_pool(name='psc', bufs=4, space='PSUM'))
    pav = ctx.enter_context(tc.tile_pool(name='pav', bufs=1, space='PSUM'))

    # ---- input DMAs ----
    # The warm tile goes first (128KB, lands ~1.5us in) so PE warm-up can
    # start immediately; the first supertile's activations follow so the PE
    # can begin projections early; weights stream right behind (per-128-row
    # chunk → subtile deps let each chunk's matmuls start as its slice lands).
    warm_sb = cpool.tile([128, 512], F16, name='warm')
    nc.sync.dma_start(out=warm_sb, in_=warm_d.ap())
    xt_sb = xpool.tile([128, 8, TOK], F16, name='xt')
    nc.sync.dma_start(out=xt_sb[:, :, 0:ST], in_=xt_d.ap()[:, :, 0:ST])

    bq_sb = cpool.tile([128, 8], F32, name='bq')
    nc.sync.dma_start(out=bq_sb, in_=bq_d.ap())
    bk_sb = cpool.tile([128, 8], F32, name='bk')
    nc.sync.dma_start(out=bk_sb, in_=bk_d.ap())

    bo_ap = bo_d.ap()
    bo_bc = cpool.tile([128, D], F32, name='bobc')
    nc.sync.dma_start(
        out=bo_bc,
        in_=bass.AP(tensor=bo_ap.tensor, offset=bo_ap.offset,
                    ap=[[0, 128], [1, D]]),
    )

    # PE warm-up: HAM un-throttles only after ~3.4us of sustained activity.
    # Run dummy matmuls on the DMA'd warm tile while the weight DMAs land so
    # the real matmul stream starts at 2.4 GHz. 14 matmuls ≈ the ~6us until
    # the first weight/activation chunks arrive (36 overshot by ~7us).
    ps_warm = pp.tile([128, 512], F32, name='ps')
    for _ in range(14):
        nc.tensor.matmul(ps_warm, lhsT=warm_sb[:, 0:128], rhs=warm_sb,
                         start=True, stop=True)

    w_sb = {}
    if QK_FP8:
        for nm, wd in (('q', wq_d), ('k', wk_d)):
            w_sb[nm] = wpool.tile([128, 4, 2, D], F8, name=f'w{nm}')
        for nm, wd in (('v', wv_d), ('o', wo_d)):
            w_sb[nm] = wpool.tile([128, 8, D], F16, name=f'w{nm}')
        x8_sb = xpool.tile([128, 4, 2, TOK], F8, name='x8')
        nc.sync.dma_start(out=x8_sb[:, :, :, 0:ST],
                          in_=x8_d.ap()[:, :, :, 0:ST])
        for nm, wd in (('q', wq_d), ('k', wk_d)):
            for c in range(4):
                nc.sync.dma_start(out=w_sb[nm][:, c, :, :], in_=wd.ap()[:, c, :, :])
        for nm, wd in (('v', wv_d), ('o', wo_d)):
            for c in range(8):
                nc.sync.dma_start(out=w_sb[nm][:, c, :], in_=wd.ap()[:, c, :])
    else:
        x8_sb = None
        for nm, wd in (('q', wq_d), ('k', wk_d), ('v', wv_d), ('o', wo_d)):
            w_sb[nm] = wpool.tile([128, 8, D], F16, name=f'w{nm}')
        for nm, wd in (('q', wq_d), ('k', wk_d), ('v', wv_d), ('o', wo_d)):
            for c in range(8):
                nc.sync.dma_start(out=w_sb[nm][:, c, :], in_=wd.ap()[:, c, :])
    # remaining activations stream behind the weights
    for s in range(1, NST):
        nc.sync.dma_start(out=xt_sb[:, :, s * ST:(s + 1) * ST],
                          in_=xt_d.ap()[:, :, s * ST:(s + 1) * ST])
        if QK_FP8:
            nc.sync.dma_start(out=x8_sb[:, :, :, s * ST:(s + 1) * ST],
                              in_=x8_d.ap()[:, :, :, s * ST:(s + 1) * ST])

    # Head grouping: 4 groups of 4 heads per block; within a group every
    # scores matmul reads Q^T/K^T at the SAME partition offset (mixing
    # offsets across matmuls that feed one PSUM bank wedges the device).
    def group_heads(g):
        parity = g % 2
        base = (g // 2) * 8
        return parity * 64, [base + parity + 2 * i for i in range(4)]

    # per-block live state for the software pipeline
    blk = {}

    def emit_scores_chain(B):
        """scores + exp + rowsum/recip/normalize + XBAR transpose for block B."""
        s, b4 = divmod(B, 4)
        t0 = b4 * 128
        qt_sb = blk[('qt', s)]
        kt_sb = blk[('kt', s)]
        e_sb = epool.tile([128, 16, 128], F16, name='e')
        at_sb = atpool.tile([128, 16, 128], F16, name='at')
        stat = epool.tile([128, 16], F16, name='stat')
        rstat = epool.tile([128, 16], F32, name='rstat')
        for g in range(4):
            off, heads = group_heads(g)
            ps_sc = psc.tile([128, 4, 128], F32, name='ps_sc')
            for i, hh in enumerate(heads):
                m = hh // 2
                nc.tensor.matmul(
                    ps_sc[:, i, :],
                    lhsT=qt_sb[off:off + 64, m, t0:t0 + 128],
                    rhs=kt_sb[off:off + 64, m, t0:t0 + 128],
                    start=True, stop=True)
            g4 = g * 4
            # softmax chain: exp on Scalar; row-sum (fp16, 2x DVE mode),
            # recip and per-head normalize on DVE.
            nc.scalar.activation(e_sb[:, g4:g4 + 4, :], ps_sc, AF.Exp)
            with nc.allow_low_precision('fp16 softmax row sums'):
                nc.vector.reduce_sum(out=stat[:, g4:g4 + 4],
                                     in_=e_sb[:, g4:g4 + 4, :], axis=AX.X)
            nc.vector.reciprocal(rstat[:, g4:g4 + 4], stat[:, g4:g4 + 4])
            nc.vector.tensor_tensor(
                out=e_sb[:, g4:g4 + 4, :], in0=e_sb[:, g4:g4 + 4, :],
                in1=rstat[:, g4:g4 + 4].to_broadcast((128, 4, 128)),
                op=OP.mult)
            if g % 2 == 1:
                # One XBAR transpose per 8 heads: in [128, 1024] -> out
                # [128, 8, 128] writes chunk j as column band j, i.e.
                # at[k, j, q] = e[q, j, k] — each head slot transposed.
                # All transposes stay on ONE ring (two rings race in the
                # shared XBAR: nondeterminism). They ride the Act ring: by
                # the time the exps ahead of them retire, the norms they
                # wait on are done, so they never block the Act queue long.
                h0 = (g - 1) * 4
                nc.scalar.dma_start_transpose(
                    out=at_sb[:, h0:h0 + 8, :],
                    in_=e_sb[:, h0:h0 + 8, :])
        blk[('at', B)] = at_sb

    def emit_av(B):
        """A@V for block B plus PSUM->SBUF repack of the head outputs."""
        s, b4 = divmod(B, 4)
        at_sb = blk.pop(('at', B))
        v_sb = blk[('v', s)]
        ps_av0 = pav.tile([128, 4, 128], F32, name='ps_av0')
        ps_av1 = pav.tile([128, 4, 128], F32, name='ps_av1')
        for g in range(4):
            off, heads = group_heads(g)
            for i, hh in enumerate(heads):
                g2 = hh // 2
                ps_av = ps_av0 if g2 < 4 else ps_av1
                nc.tensor.matmul(
                    ps_av[off:off + 64, g2 % 4, :],
                    lhsT=v_sb[:, b4, hh * 64:(hh + 1) * 64],
                    rhs=at_sb[:, g * 4 + i, :],
                    start=True, stop=True)
        ot_sb = otpool.tile([128, 8, 128], F16, name='ot')
        nc.scalar.copy(ot_sb[:, 0:4, :], ps_av0)
        nc.scalar.copy(ot_sb[:, 4:8, :], ps_av1)
        blk[('ot', B)] = ot_sb

    def emit_wo(B):
        """output projection + bias for block B, store to DRAM."""
        ot_sb = blk.pop(('ot', B))
        tok0 = B * 128
        for nh2 in range(2):
            ps = pp.tile([128, 512], F32, name='ps')
            for c in range(8):
                nc.tensor.matmul(
                    ps, lhsT=ot_sb[:, c, :],
                    rhs=w_sb['o'][:, c, nh2 * 512:(nh2 + 1) * 512],
                    start=(c == 0), stop=(c == 7))
            out_sb = opool.tile([128, 512], F32, name='outsb')
            nc.vector.tensor_tensor(
                out=out_sb, in0=ps,
                in1=bo_bc[:, nh2 * 512:(nh2 + 1) * 512], op=OP.add)
            # Stores go on the SP ring with the transposes: a waiting HWDGE
            # DMA blocks its issuing engine's whole queue, and the Scalar
            # queue carries latency-critical exp/copy work.
            nc.sync.dma_start(
                out=out_d.ap()[tok0:tok0 + 128, nh2 * 512:(nh2 + 1) * 512],
                in_=out_sb)

    for s in range(NST):
        xs = xt_sb[:, :, s * ST:(s + 1) * ST]

        # ---- projections ----
        qt_sb = qkv.tile([128, 8, ST], F16, name='qt')
        kt_sb = qkv.tile([128, 8, ST], F16, name='kt')
        v_sb = qkv.tile([128, 4, D], F16, name='v')
        blk[('qt', s)] = qt_sb
        blk[('kt', s)] = kt_sb
        blk[('v', s)] = v_sb

        if QK_FP8:
            # DoubleRow fp8: 2 k-rows packed per partition, 4 chunks of 256.
            x8s = x8_sb[:, :, :, s * ST:(s + 1) * ST]
            DR = mybir.MatmulPerfMode.DoubleRow
            for nm, t_sb, b_sb, sc in (('q', qt_sb, bq_sb, SCALE),
                                       ('k', kt_sb, bk_sb, 1.0)):
                for m in range(8):
                    ps = pp.tile([128, 512], F32, name='ps')
                    for c in range(4):
                        nc.tensor.matmul(
                            ps, lhsT=w_sb[nm][:, c, :, m * 128:(m + 1) * 128],
                            rhs=x8s[:, c, :, :], start=(c == 0), stop=(c == 3),
                            perf_mode=DR)
                    nc.scalar.activation(t_sb[:, m, :], ps, AF.Identity,
                                         bias=b_sb[:, m:m + 1], scale=sc)
        else:
            for m in range(8):
                ps = pp.tile([128, 512], F32, name='ps')
                for c in range(8):
                    nc.tensor.matmul(ps, lhsT=w_sb['q'][:, c, m * 128:(m + 1) * 128],
                                     rhs=xs[:, c, :], start=(c == 0), stop=(c == 7))
                nc.scalar.activation(qt_sb[:, m, :], ps, AF.Identity,
                                     bias=bq_sb[:, m:m + 1], scale=SCALE)
            for m in range(8):
                ps = pp.tile([128, 512], F32, name='ps')
                for c in range(8):
                    nc.tensor.matmul(ps, lhsT=w_sb['k'][:, c, m * 128:(m + 1) * 128],
                                     rhs=xs[:, c, :], start=(c == 0), stop=(c == 7))
                nc.scalar.activation(kt_sb[:, m, :], ps, AF.Identity,
                                     bias=bk_sb[:, m:m + 1], scale=1.0)

        # scores/softmax for this supertile's first block can start as soon
        # as Q^T/K^T exist — emit before the V projection so the V matmuls
        # hide the softmax chain.
        emit_scores_chain(4 * s)

        # V projection: bias bv is NOT added here — softmax rows sum to 1, so
        # A@(V + 1*bv) = A@V + 1*bv, and bv@Wo is folded into bo on the host.
        # The drain is then a plain PSUM->SBUF copy on the Scalar engine.
        for tch in range(4):
            for nh2 in range(2):
                ps = pp.tile([128, 512], F32, name='ps')
                for c in range(8):
                    nc.tensor.matmul(
                        ps, lhsT=xs[:, c, tch * 128:(tch + 1) * 128],
                        rhs=w_sb['v'][:, c, nh2 * 512:(nh2 + 1) * 512],
                        start=(c == 0), stop=(c == 7))
                nc.scalar.copy(v_sb[:, tch, nh2 * 512:(nh2 + 1) * 512], ps)

        # ---- attention blocks, two-block software pipeline ----
        # iter B: scores/softmax(B+1) | A@V(B-1) | Wo(B-2). The softmax +
        # XBAR-transpose chain for a block gets a full iteration (~5us) of
        # PE work as cover before its A@V consumes it.
        for b4 in range(4):
            B = 4 * s + b4
            if b4 < 3:
                emit_scores_chain(B + 1)
            if B > 0:
                emit_av(B - 1)
            if B > 1:
                emit_wo(B - 2)

    emit_av(NBLK - 1)
    emit_wo(NBLK - 2)
    emit_wo(NBLK - 1)


_NC_CACHE = []


def _get_nc():
    if not _NC_CACHE:
        _NC_CACHE.append(build_bass())
    return _NC_CACHE[0]


def shard_inputs(x, Wq, bq, Wk, bk, Wv, bv, Wo, bo):
    x = np.asarray(x, dtype=np.float32)
    B, S, _ = x.shape
    xf = np.ascontiguousarray(x.reshape(B * S, D))
    assert B * S == N_CORES * TOK

    def packw(W):
        # [d_in, d_out] -> [p, c, d_out] with d_in = c*128 + p
        return np.ascontiguousarray(
            np.asarray(W, dtype=np.float16).reshape(8, 128, D).transpose(1, 0, 2))

    F8NP = mybir.dt.np(F8)

    def packw8(W):
        # [d_in, d_out] -> [p, c, kt, d_out] with d_in = c*256 + kt*128 + p
        return np.ascontiguousarray(
            np.asarray(W, dtype=np.float32).astype(F8NP)
            .reshape(4, 2, 128, D).transpose(2, 0, 1, 3))

    shared = {
        'wq': packw8(Wq) if QK_FP8 else packw(Wq),
        'wk': packw8(Wk) if QK_FP8 else packw(Wk),
        'wv': packw(Wv),
        'wo': packw(Wo),
        'warm': np.full((128, 512), 0.5, dtype=np.float16),
        'bq': np.ascontiguousarray(
            (np.asarray(bq, dtype=np.float32) * SCALE).reshape(8, 128).T),
        'bk': np.ascontiguousarray(
            np.asarray(bk, dtype=np.float32).reshape(8, 128).T),
        # bv folded through the attention (softmax rows sum to 1):
        # out = A@(V+1*bv)@Wo + bo = A@V@Wo + (bv@Wo + bo)
        'bv': np.ascontiguousarray(np.asarray(bv, dtype=np.float32).reshape(1, D)),
        'bo': np.ascontiguousarray(
            (np.asarray(bv, dtype=np.float32) @ np.asarray(Wo, dtype=np.float32)
             + np.asarray(bo, dtype=np.float32)).reshape(1, D)),
    }
    in_maps = []
    for c in range(N_CORES):
        xs = xf[c * TOK:(c + 1) * TOK, :]  # [TOK, D]
        # [p, c, t] with d_in = c*128 + p
        xt = np.ascontiguousarray(
            xs.T.astype(np.float16).reshape(8, 128, TOK).transpose(1, 0, 2))
        m = {'xt': xt, **shared}
        if QK_FP8:
            # [p, c, kt, t] with d_in = c*256 + kt*128 + p
            m['x8'] = np.ascontiguousarray(
                xs.T.astype(F8NP).reshape(4, 2, 128, TOK).transpose(2, 0, 1, 3))
        in_maps.append(m)
    return (B, S), in_maps


def run(inputs, **spmd_kwargs):
    (B, S), in_maps = shard_inputs(**inputs)
    nc = _get_nc()
    res = run_bass_kernel_spmd(nc, in_maps, list(range(N_CORES)), **spmd_kwargs)
    out = np.concatenate([res.results[c]['out'] for c in range(N_CORES)], axis=0)
    return out.reshape(B, S, D), res


def kernel(x, Wq, bq, Wk, bk, Wv, bv, Wo, bo):
    out, _ = run(dict(x=x, Wq=Wq, bq=bq, Wk=Wk, bk=bk,
                      Wv=Wv, bv=bv, Wo=Wo, bo=bo))
    return out
